# revision 28
# baseline (speedup 1.0000x reference)
"""Trainium2 Bass kernel for ConsistentSelfAttentionTile.

Reference semantics: T=449 overlapping 64-token tiles; each tile attends to
352 KV tokens = 288 sampled (from a 9x replication of the tile) + the tile
itself; outputs overlap-add, then divide by overlap counts.

Algebraic collapse (same as the verified baseline):
  * rep[:, idx, :] == tile[:, idx % 64, :], so sampled KV tokens are tile
    rows with multiplicities m_t[w] = 1 + #{s : idx[t,s] % 64 == w}.
  * All per-tile 64x64 score blocks are diagonal blocks of one banded
    512x512 score matrix S = Q K^T (band |i-j| <= 63).
  * With E^T = exp(S^T + kb - 40), Cm[j,t] = m_t[j-t] (banded), the
    tile-softmax + overlap-add + count-divide collapses to
        Z = Cm^T E^T; W = mask/(counts * Z); U = Cm W; out = (E^T*U)^T V
    The constant -40 shift (vs a per-row max) cancels exactly; kb[j] =
    K[j].bq carries the Q bias (host-precomputed as x_band @ (Wk^T bq),
    exact), so Q/K projections are bias-free on device.
  * bk drops exactly (softmax-invariant); bv adds on the HOST to the
    returned output (attention rows sum to exactly 1), so V is bias-free.

v4 design notes (baseline 37.9us -> v3 31.3us -> this):
  * Input DMA: 9 unchained FIFO pieces on the two HWDGE rings (sync +
    scalar), ordered by need-time; x/wq/wk ship in k-chunk halves so the
    projections start accumulating as soon as each half lands. Cm/CmT
    ship as uint8 and cast to bf16 by a GPSIMD (SWDGE) DMA on its own
    ring. Output is fp16 (host casts back) in 2 halves on 2 rings.
  * PE runs at 1.2 GHz until the HAM sees ~3.4us of CONTINUOUS activity
    (idle gaps re-throttle): dependency-free warm-up matmuls run under
    the DMA lead-in and between arrival-gated phases.
  * PSUM: one OPEN accumulation group per bank at a time (start=True
    clobbers sibling has_written). 8 banks: KT 2, QT->st->Z->U rotate 2,
    V 2, out 2 (shared with warm-up).
  * exp bias rides per-partition (kb - 40); 1/Z via the ~5x-faster
    reciprocal_approx_fast; drains split across ACT and DVE.

Sharding: 8 cores = 2 batches x 4 row-chunks of 128 output rows, each core
fully independent on a 256-column band of its batch's sequence.
"""

import os
import sys

import numpy as np

try:
    import ml_dtypes
except ImportError:
    ml_dtypes = None

for _p in ("/opt/trn_rl_repo",):
    if _p not in sys.path and os.path.isdir(_p):
        sys.path.insert(0, _p)

B, N, C, W = 2, 512, 512, 64
T = N - W + 1          # 449 tiles
RCH = 128              # output rows per core
NCORES = 8
BAND = 256             # per-core j/t band width (columns [r0-64, r0+192))
KC = C // 128          # 4 contraction chunks
JC = BAND // 128       # 2 band chunks
ESHIFT = -40.0         # constant exp shift (cancels exactly; keeps the
                       # activation-table inputs in the proven-negative range)

# blob layout (fp16 columns; kb is a [128,2] fp32 bitcast view). Order is
# the DMA piece order: sync ring x+kb | G halves, scalar ring wv halves;
# cm/cmt (uint8) and mw ride the GPSIMD SWDGE ring. G = Wq^T Wk folds
# both score-path weight matrices into one (S = x_q G x_band^T), so no K
# projection exists on device at all.
OFF_XT = 0                       # [128, 4, 256] x^T band
OFF_KB = OFF_XT + KC * BAND      # [128, 2] fp32 exp bias kb[j]+ESHIFT
OFF_G = OFF_KB + 4               # [128, 4, 512] G = Wq^T Wk (d-chunk-major)
OFF_WVT = OFF_G + KC * C         # [128, 4, 512]
OFF_MW = OFF_WVT + KC * C        # [128, 2, 128] fp16 mask/counts
F16 = OFF_MW + JC * RCH

NCM = 2 * JC * BAND              # cm | cmt as uint8, cast to bf16 on-chip

_CACHE = {}


def _slim_drain_and_barrier(self, tick_clock, wait_clock):
    """Cheaper TileContext exit: final drain covers only DMA-queue
    completion, then one sem-only barrier + semaphore reset."""
    from concourse.vector_clock import ScopedClock, VectorClock
    from concourse.tile_scheduler import dmasw_start_idx, N_PROCS

    g = tick_clock.global_clock
    dma_clock = VectorClock()
    for idx in range(dmasw_start_idx, N_PROCS):
        t = g.peek_next(idx) - 1
        if t > 0:
            dma_clock.require_at_least(idx, t)
    drain_inst = self.nc.sync.drain()
    wait_clock.add_sem_waits(drain_inst.ins, ScopedClock({None: dma_clock}))
    self.nc.all_engine_barrier(sem_only=True)
    popped = self.nc._tile_sem_poison_stack.pop()
    assert popped is self._sem_poison
    self.nc.clear_and_free_semaphores(list(self.sems.allocated().values()))


def _build_program():
    import concourse.bacc as bacc
    import concourse.mybir as mybir
    import concourse.tile as tile

    fp16 = mybir.dt.float16
    uint8 = mybir.dt.uint8
    # Skip Bass's preamble all-engine barrier (Tile sems carry all real
    # deps), so the input DMA issues ~5us earlier.
    orig_aeb = bacc.Bacc.all_engine_barrier

    def _noop_aeb(self, *, sem_only=False):
        return None

    bacc.Bacc.all_engine_barrier = _noop_aeb
    try:
        nc = bacc.Bacc("TRN2", target_bir_lowering=False, debug=False)
    finally:
        bacc.Bacc.all_engine_barrier = orig_aeb

    blob_d = nc.declare_dram_parameter("blob", [128, F16], fp16, isOutput=False)
    cm_d = nc.declare_dram_parameter("cmu8", [128, NCM], uint8, isOutput=False)
    out_d = nc.declare_dram_parameter("out", [RCH, C], fp16, isOutput=True)

    orig_dab = tile.TileContext._drain_and_barrier
    tile.TileContext._drain_and_barrier = _slim_drain_and_barrier
    try:
        _emit_body(nc, tile, mybir, blob_d, cm_d, out_d)
    finally:
        tile.TileContext._drain_and_barrier = orig_dab

    nc.compile()
    return nc


def _emit_body(nc, tile, mybir, blob_d, cm_d, out_d):
    fp32 = mybir.dt.float32
    fp16 = mybir.dt.float16
    bf16 = mybir.dt.bfloat16
    AFT = mybir.ActivationFunctionType

    with tile.TileContext(nc) as tc:
        with (
            tc.tile_pool(name="consts", bufs=1) as consts,
            tc.tile_pool(name="work", bufs=1) as work,
            tc.tile_pool(name="psum", bufs=1, space="PSUM") as psum,
        ):
            blob = consts.tile([128, F16], fp16)
            cmu_sb = consts.tile([128, NCM], mybir.dt.uint8)
            cmx_sb = consts.tile([128, NCM], bf16)
            dum_src = work.tile([128, 256], fp16)
            nc.gpsimd.memset(dum_src, 0.0)

            # ---- input DMA: unchained FIFO pieces; each ring drains in
            # instruction order and the SDMA engines round-robin rings.
            sync_pieces = [
                (OFF_XT, OFF_G),                 # x + kb
                (OFF_G, OFF_G + 2 * C),          # G k0,k1
                (OFF_G + 2 * C, OFF_WVT),        # G k2,k3
            ]
            scalar_pieces = [
                (OFF_WVT, OFF_WVT + 2 * C),      # wv k0,k1
                (OFF_WVT + 2 * C, OFF_MW),       # wv k2,k3
            ]
            for lo, hi in sync_pieces:
                nc.sync.dma_start(out=blob[:, lo:hi], in_=blob_d[:, lo:hi])
            for lo, hi in scalar_pieces:
                nc.scalar.dma_start(out=blob[:, lo:hi], in_=blob_d[:, lo:hi])
            # cm/cmt (uint8, cast to bf16 on-chip by the otherwise-idle
            # GPSIMD) and mw ride the SWDGE ring
            nc.gpsimd.dma_start(out=cmu_sb, in_=cm_d[:])
            nc.gpsimd.dma_start(
                out=blob[:, OFF_MW:F16], in_=blob_d[:, OFF_MW:F16])
            nc.gpsimd.tensor_copy(out=cmx_sb, in_=cmu_sb)

            xt_v = blob[:, OFF_XT:OFF_XT + KC * BAND].rearrange(
                "p (k j) -> p k j", k=KC)

            def xt(k):
                return xt_v[:, k, :]

            g_sb = blob[:, OFF_G:OFF_G + KC * C].rearrange(
                "p (k j) -> p k j", k=KC)
            wvt_v = blob[:, OFF_WVT:OFF_WVT + KC * C].rearrange(
                "p (k j) -> p k j", k=KC)

            def wvt(k):
                return wvt_v[:, k, :]
            kb = blob[:, OFF_KB:OFF_KB + 4].bitcast(fp32)
            mw = blob[:, OFF_MW:OFF_MW + JC * RCH].rearrange(
                "p (k r) -> p k r", k=JC)
            cm = cmx_sb[:, 0:JC * BAND].rearrange("p (k t) -> p k t", k=JC)
            cmt = cmx_sb[:, JC * BAND:NCM].rearrange("p (k j) -> p k j", k=JC)

            # ---- PE warm-up: dependency-free matmuls keep the HAM clock
            # gate fed while DMA streams (PE is 1.2 GHz until ~3.4us of
            # continuous activity; gaps re-throttle).
            ps_dum = psum.tile([128, 256], fp32, tag="ps_o", bufs=2)

            def dummies(n):
                for _ in range(n):
                    nc.tensor.matmul(
                        ps_dum, lhsT=dum_src[:, 0:128], rhs=dum_src,
                        start=True, stop=True,
                    )

            dummies(22)

            # ---- V[jc][j 128, c 512] = x_band Wv^T, k-order matching
            # the wv DMA halves (PE's first real work; doubles as warm-up)
            ps_v = [psum.tile([128, C], fp32, tag="ps_v", bufs=2,
                              name=f"ps_v{jc}")
                    for jc in range(JC)]
            for k in range(KC):
                for jc in range(JC):
                    nc.tensor.matmul(
                        ps_v[jc],
                        lhsT=xt(k)[:, jc * 128:(jc + 1) * 128],
                        rhs=wvt(k),
                        start=(k == 0),
                        stop=(k == KC - 1),
                    )
            dummies(2)

            # ---- HH[m][e 128, r 128] = G^T x_q^T (the whole score path's
            # only projection). 4 banks -> 4 independent groups, k-major
            # so each G half releases 8 matmuls.
            ps_hh = [psum.tile([128, RCH], fp32, tag="ps_q", bufs=4,
                               name=f"ps_hh{m}")
                     for m in range(KC)]
            hh_sb = work.tile([128, KC, RCH], fp16)
            for k in range(KC):
                for m in range(KC):
                    nc.tensor.matmul(
                        ps_hh[m],
                        lhsT=g_sb[:, k, m * 128:(m + 1) * 128],
                        rhs=xt(k)[:, 64:64 + RCH],
                        start=(k == 0),
                        stop=(k == KC - 1),
                    )
            for m in range(KC):
                if m % 2 == 0:
                    nc.scalar.copy(out=hh_sb[:, m, :], in_=ps_hh[m])
                else:
                    nc.vector.tensor_copy(out=hh_sb[:, m, :], in_=ps_hh[m])

            # ---- V drains (out needs them later than st needs hh)
            v_sb = work.tile([128, JC, C], fp16)
            for jc in range(JC):
                nc.vector.tensor_copy(
                    out=v_sb[:, jc, 0:256], in_=ps_v[jc][:, 0:256])
                nc.scalar.copy(
                    out=v_sb[:, jc, 256:512], in_=ps_v[jc][:, 256:512])

            # ---- st[jc][j 128, r 128] = K Q^T (scores TRANSPOSED), then
            # E^T = exp(st + kb[j] - 40) straight to SBUF bf16.
            et_sb = work.tile([128, JC, RCH], bf16)
            ps_st = psum.tile([128, JC * RCH], fp32, tag="ps_q", bufs=4)
            for jc in range(JC):
                for m in range(KC):
                    nc.tensor.matmul(
                        ps_st[:, jc * RCH:(jc + 1) * RCH],
                        lhsT=xt(m)[:, jc * 128:(jc + 1) * 128],
                        rhs=hh_sb[:, m, :],
                        start=(m == 0),
                        stop=(m == KC - 1),
                        skip_group_check=True,
                    )
                nc.scalar.activation(
                    out=et_sb[:, jc, :], in_=ps_st[:, jc * RCH:(jc + 1) * RCH],
                    func=AFT.Exp, bias=kb[:, jc:jc + 1], scale=1.0,
                )
                dummies(2)

            # ---- Z[tch][t 128, r 128] = Cm^T E^T;  W = mw / Z
            w_sb = work.tile([128, JC, RCH], bf16)
            rz = work.tile([128, JC, RCH], fp32)
            ps_z = [psum.tile([128, RCH], fp32, tag="ps_q", bufs=4,
                              name=f"ps_z{tch}")
                    for tch in range(JC)]
            for jc in range(JC):          # jc-major: both tch advance per exp
                for tch in range(JC):
                    nc.tensor.matmul(
                        ps_z[tch],
                        lhsT=cm[:, jc, tch * 128:(tch + 1) * 128],
                        rhs=et_sb[:, jc, :],
                        start=(jc == 0),
                        stop=(jc == JC - 1),
                    )
                dummies(2)
            for tch in range(JC):
                nc.vector.reciprocal_approx_fast(
                    out=rz[:, tch, :], in_=ps_z[tch])
                nc.vector.tensor_mul(
                    w_sb[:, tch, :], rz[:, tch, :], mw[:, tch, :]
                )

            # ---- U[jc][j 128, r 128] = Cm W;  A = E^T * U
            a_sb = work.tile([128, JC, RCH], fp16)
            ps_u = [psum.tile([128, RCH], fp32, tag="ps_q", bufs=4,
                              name=f"ps_u{jc}")
                    for jc in range(JC)]
            for tch in range(JC):         # tch-major: both jc advance per W
                for jc in range(JC):
                    nc.tensor.matmul(
                        ps_u[jc],
                        lhsT=cmt[:, tch, jc * 128:(jc + 1) * 128],
                        rhs=w_sb[:, tch, :],
                        start=(tch == 0),
                        stop=(tch == JC - 1),
                    )
                dummies(2)
            for jc in range(JC):
                nc.vector.tensor_mul(
                    a_sb[:, jc, :], ps_u[jc], et_sb[:, jc, :]
                )

            # ---- out[r 128, c 512] = A^T V (bv adds on the host), fp16,
            # in 2 column halves so the first DMA overlaps the second.
            o_sb = work.tile([128, C], fp16)
            for ch in range(2):
                cs = ch * (C // 2)
                ps_o = psum.tile([128, C // 2], fp32, tag="ps_o", bufs=2,
                                 name=f"ps_o{ch}")
                for jc in range(JC):
                    nc.tensor.matmul(
                        ps_o,
                        lhsT=a_sb[:, jc, :],
                        rhs=v_sb[:, jc, cs:cs + C // 2],
                        start=(jc == 0),
                        stop=(jc == JC - 1),
                    )
                if ch == 0:
                    nc.vector.tensor_copy(
                        out=o_sb[:, cs:cs + C // 2], in_=ps_o)
                    nc.sync.dma_start(
                        out=out_d[:, cs:cs + C // 2],
                        in_=o_sb[:, cs:cs + C // 2])
                else:
                    nc.scalar.copy(out=o_sb[:, cs:cs + C // 2], in_=ps_o)
                    nc.scalar.dma_start(
                        out=out_d[:, cs:cs + C // 2],
                        in_=o_sb[:, cs:cs + C // 2])


# revision 30
# speedup vs baseline: 1.0515x; 1.0515x over previous
"""Trainium2 Bass kernel for ConsistentSelfAttentionTile.

Reference semantics: T=449 overlapping 64-token tiles; each tile attends to
352 KV tokens = 288 sampled (from a 9x replication of the tile) + the tile
itself; outputs overlap-add, then divide by overlap counts.

Algebraic collapse (same as the verified baseline):
  * rep[:, idx, :] == tile[:, idx % 64, :], so sampled KV tokens are tile
    rows with multiplicities m_t[w] = 1 + #{s : idx[t,s] % 64 == w}.
  * All per-tile 64x64 score blocks are diagonal blocks of one banded
    512x512 score matrix S = Q K^T (band |i-j| <= 63).
  * With E^T = exp(S^T + kb - 40), Cm[j,t] = m_t[j-t] (banded), the
    tile-softmax + overlap-add + count-divide collapses to
        Z = Cm^T E^T; W = mask/(counts * Z); U = Cm W; out = (E^T*U)^T V
    The constant -40 shift (vs a per-row max) cancels exactly; kb[j] =
    K[j].bq carries the Q bias (host-precomputed as x_band @ (Wk^T bq),
    exact), so Q/K projections are bias-free on device.
  * bk drops exactly (softmax-invariant); bv adds on the HOST to the
    returned output (attention rows sum to exactly 1), so V is bias-free.

v4 design notes (baseline 37.9us -> v3 31.3us -> this):
  * Input DMA: 9 unchained FIFO pieces on the two HWDGE rings (sync +
    scalar), ordered by need-time; x/wq/wk ship in k-chunk halves so the
    projections start accumulating as soon as each half lands. Cm/CmT
    ship as uint8 and cast to bf16 by a GPSIMD (SWDGE) DMA on its own
    ring. Output is fp16 (host casts back) in 2 halves on 2 rings.
  * PE runs at 1.2 GHz until the HAM sees ~3.4us of CONTINUOUS activity
    (idle gaps re-throttle): dependency-free warm-up matmuls run under
    the DMA lead-in and between arrival-gated phases.
  * PSUM: one OPEN accumulation group per bank at a time (start=True
    clobbers sibling has_written). 8 banks: KT 2, QT->st->Z->U rotate 2,
    V 2, out 2 (shared with warm-up).
  * exp bias rides per-partition (kb - 40); 1/Z via the ~5x-faster
    reciprocal_approx_fast; drains split across ACT and DVE.

Sharding: 8 cores = 2 batches x 4 row-chunks of 128 output rows, each core
fully independent on a 256-column band of its batch's sequence.
"""

import os
import sys

import numpy as np

try:
    import ml_dtypes
except ImportError:
    ml_dtypes = None

for _p in ("/opt/trn_rl_repo",):
    if _p not in sys.path and os.path.isdir(_p):
        sys.path.insert(0, _p)

B, N, C, W = 2, 512, 512, 64
T = N - W + 1          # 449 tiles
RCH = 128              # output rows per core
NCORES = 8
BAND = 256             # per-core j/t band width (columns [r0-64, r0+192))
KC = C // 128          # 4 contraction chunks
JC = BAND // 128       # 2 band chunks
ESHIFT = -40.0         # constant exp shift (cancels exactly; keeps the
                       # activation-table inputs in the proven-negative range)

# blob layout (fp16 columns; kb is a [128,2] fp32 bitcast view). Order is
# the DMA piece order: sync ring x+kb | G halves, scalar ring wv halves;
# cm/cmt (uint8) and mw ride the GPSIMD SWDGE ring. G = Wq^T Wk folds
# both score-path weight matrices into one (S = x_q G x_band^T), so no K
# projection exists on device at all.
OFF_XT = 0                       # [128, 4, 256] x^T band
OFF_KB = OFF_XT + KC * BAND      # [128, 2] fp32 exp bias kb[j]+ESHIFT
OFF_G = OFF_KB + 4               # [128, 4, 512] G = Wq^T Wk (d-chunk-major)
OFF_WVT = OFF_G + KC * C         # [128, 4, 512]
OFF_MW = OFF_WVT + KC * C        # [128, 2, 128] fp16 mask/counts
F16 = OFF_MW + JC * RCH

NCM = 2 * JC * BAND              # cm | cmt as uint8, cast to bf16 on-chip

_CACHE = {}


def _slim_drain_and_barrier(self, tick_clock, wait_clock):
    """Cheaper TileContext exit: final drain covers only DMA-queue
    completion, then one sem-only barrier + semaphore reset."""
    from concourse.vector_clock import ScopedClock, VectorClock
    from concourse.tile_scheduler import dmasw_start_idx, N_PROCS

    g = tick_clock.global_clock
    dma_clock = VectorClock()
    for idx in range(dmasw_start_idx, N_PROCS):
        t = g.peek_next(idx) - 1
        if t > 0:
            dma_clock.require_at_least(idx, t)
    drain_inst = self.nc.sync.drain()
    wait_clock.add_sem_waits(drain_inst.ins, ScopedClock({None: dma_clock}))
    self.nc.all_engine_barrier(sem_only=True)
    popped = self.nc._tile_sem_poison_stack.pop()
    assert popped is self._sem_poison
    self.nc.clear_and_free_semaphores(list(self.sems.allocated().values()))


def _build_program():
    import concourse.bacc as bacc
    import concourse.mybir as mybir
    import concourse.tile as tile

    fp16 = mybir.dt.float16
    uint8 = mybir.dt.uint8
    # Skip Bass's preamble all-engine barrier (Tile sems carry all real
    # deps), so the input DMA issues ~5us earlier.
    orig_aeb = bacc.Bacc.all_engine_barrier

    def _noop_aeb(self, *, sem_only=False):
        return None

    bacc.Bacc.all_engine_barrier = _noop_aeb
    try:
        nc = bacc.Bacc("TRN2", target_bir_lowering=False, debug=False)
    finally:
        bacc.Bacc.all_engine_barrier = orig_aeb

    blob_d = nc.declare_dram_parameter("blob", [128, F16], fp16, isOutput=False)
    cm_d = nc.declare_dram_parameter("cmu8", [128, NCM], uint8, isOutput=False)
    out_d = nc.declare_dram_parameter("out", [RCH, C], fp16, isOutput=True)

    orig_dab = tile.TileContext._drain_and_barrier
    tile.TileContext._drain_and_barrier = _slim_drain_and_barrier
    try:
        _emit_body(nc, tile, mybir, blob_d, cm_d, out_d)
    finally:
        tile.TileContext._drain_and_barrier = orig_dab

    nc.compile()
    return nc


def _emit_body(nc, tile, mybir, blob_d, cm_d, out_d):
    fp32 = mybir.dt.float32
    fp16 = mybir.dt.float16
    bf16 = mybir.dt.bfloat16
    AFT = mybir.ActivationFunctionType

    with tile.TileContext(nc) as tc:
        with (
            tc.tile_pool(name="consts", bufs=1) as consts,
            tc.tile_pool(name="work", bufs=1) as work,
            tc.tile_pool(name="psum", bufs=1, space="PSUM") as psum,
        ):
            blob = consts.tile([128, F16], fp16)
            cmu_sb = consts.tile([128, NCM], mybir.dt.uint8)
            cmx_sb = consts.tile([128, NCM], bf16)
            dum_src = work.tile([128, 256], fp16)
            nc.gpsimd.memset(dum_src, 0.0)

            # ---- input DMA: unchained FIFO pieces; each ring drains in
            # instruction order and the SDMA engines round-robin rings.
            sync_pieces = [
                (OFF_XT, OFF_G),                 # x + kb
                (OFF_G, OFF_G + 2 * C),          # G k0,k1
                (OFF_WVT + 2 * C, OFF_MW),       # wv k2,k3 (longest slack)
            ]
            scalar_pieces = [
                (OFF_WVT, OFF_WVT + 2 * C),      # wv k0,k1
                (OFF_G + 2 * C, OFF_WVT),        # G k2,k3 (score chain gate)
            ]
            for lo, hi in sync_pieces:
                nc.sync.dma_start(out=blob[:, lo:hi], in_=blob_d[:, lo:hi])
            for lo, hi in scalar_pieces:
                nc.scalar.dma_start(out=blob[:, lo:hi], in_=blob_d[:, lo:hi])
            # cm/cmt (uint8, cast to bf16 on-chip by the otherwise-idle
            # GPSIMD) and mw ride the SWDGE ring
            nc.gpsimd.dma_start(out=cmu_sb, in_=cm_d[:])
            nc.gpsimd.dma_start(
                out=blob[:, OFF_MW:F16], in_=blob_d[:, OFF_MW:F16])
            nc.gpsimd.tensor_copy(out=cmx_sb, in_=cmu_sb)

            xt_v = blob[:, OFF_XT:OFF_XT + KC * BAND].rearrange(
                "p (k j) -> p k j", k=KC)

            def xt(k):
                return xt_v[:, k, :]

            g_sb = blob[:, OFF_G:OFF_G + KC * C].rearrange(
                "p (k j) -> p k j", k=KC)
            wvt_v = blob[:, OFF_WVT:OFF_WVT + KC * C].rearrange(
                "p (k j) -> p k j", k=KC)

            def wvt(k):
                return wvt_v[:, k, :]
            kb = blob[:, OFF_KB:OFF_KB + 4].bitcast(fp32)
            mw = blob[:, OFF_MW:OFF_MW + JC * RCH].rearrange(
                "p (k r) -> p k r", k=JC)
            cm = cmx_sb[:, 0:JC * BAND].rearrange("p (k t) -> p k t", k=JC)
            cmt = cmx_sb[:, JC * BAND:NCM].rearrange("p (k j) -> p k j", k=JC)

            # ---- PE warm-up: dependency-free matmuls keep the HAM clock
            # gate fed while DMA streams (PE is 1.2 GHz until ~3.4us of
            # continuous activity; gaps re-throttle).
            ps_dum = psum.tile([128, 256], fp32, tag="ps_o", bufs=2)

            def dummies(n):
                for _ in range(n):
                    nc.tensor.matmul(
                        ps_dum, lhsT=dum_src[:, 0:128], rhs=dum_src,
                        start=True, stop=True,
                    )

            dummies(14)

            # ---- V[jc][j 128, c 512] = x_band Wv^T, k-order matching
            # the wv DMA halves (PE's first real work; doubles as warm-up)
            ps_v = [psum.tile([128, C], fp32, tag="ps_v", bufs=2,
                              name=f"ps_v{jc}")
                    for jc in range(JC)]
            def v_k(k, start, stop):
                for jc in range(JC):
                    nc.tensor.matmul(
                        ps_v[jc],
                        lhsT=xt(k)[:, jc * 128:(jc + 1) * 128],
                        rhs=wvt(k),
                        start=start, stop=stop,
                    )

            v_k(0, True, False)       # wv01 is scalar's first piece
            v_k(1, False, False)
            dummies(2)

            # ---- HH[m][e 128, r 128] = G^T x_q^T (the whole score path's
            # only projection). 4 banks -> 4 independent groups, k-major
            # so each G half releases 8 matmuls.
            ps_hh = [psum.tile([128, RCH], fp32, tag="ps_q", bufs=4,
                               name=f"ps_hh{m}")
                     for m in range(KC)]
            hh_sb = work.tile([128, KC, RCH], fp16)
            for k in (2, 3, 0, 1):    # G23 lands before G01
                for m in range(KC):
                    nc.tensor.matmul(
                        ps_hh[m],
                        lhsT=g_sb[:, k, m * 128:(m + 1) * 128],
                        rhs=xt(k)[:, 64:64 + RCH],
                        start=(k == 2),
                        stop=(k == 1),
                    )
            for m in range(KC):
                if m % 2 == 0:
                    nc.scalar.copy(out=hh_sb[:, m, :], in_=ps_hh[m])
                else:
                    nc.vector.tensor_copy(out=hh_sb[:, m, :], in_=ps_hh[m])

            # ---- st[jc][j 128, r 128] = K Q^T (scores TRANSPOSED), then
            # E^T = exp(st + kb[j] - 40) straight to SBUF bf16.
            et_sb = work.tile([128, JC, RCH], bf16)
            ps_st = psum.tile([128, JC * RCH], fp32, tag="ps_q", bufs=4)
            for jc in range(JC):
                for m in range(KC):
                    nc.tensor.matmul(
                        ps_st[:, jc * RCH:(jc + 1) * RCH],
                        lhsT=xt(m)[:, jc * 128:(jc + 1) * 128],
                        rhs=hh_sb[:, m, :],
                        start=(m == 0),
                        stop=(m == KC - 1),
                        skip_group_check=True,
                    )
                nc.scalar.activation(
                    out=et_sb[:, jc, :], in_=ps_st[:, jc * RCH:(jc + 1) * RCH],
                    func=AFT.Exp, bias=kb[:, jc:jc + 1], scale=1.0,
                )
                dummies(2)

            v_k(2, False, False)      # wv23 fills the exp/Z round-trips

            # ---- Z[tch][t 128, r 128] = Cm^T E^T;  W = mw / Z
            w_sb = work.tile([128, JC, RCH], bf16)
            rz = work.tile([128, JC, RCH], fp32)
            ps_z = [psum.tile([128, RCH], fp32, tag="ps_q", bufs=4,
                              name=f"ps_z{tch}")
                    for tch in range(JC)]
            for jc in range(JC):          # jc-major: both tch advance per exp
                for tch in range(JC):
                    nc.tensor.matmul(
                        ps_z[tch],
                        lhsT=cm[:, jc, tch * 128:(tch + 1) * 128],
                        rhs=et_sb[:, jc, :],
                        start=(jc == 0),
                        stop=(jc == JC - 1),
                    )
                dummies(2)
            for tch in range(JC):
                nc.vector.reciprocal_approx_fast(
                    out=rz[:, tch, :], in_=ps_z[tch])
                nc.vector.tensor_mul(
                    w_sb[:, tch, :], rz[:, tch, :], mw[:, tch, :]
                )

            # ---- U[jc][j 128, r 128] = Cm W;  A = E^T * U
            a_sb = work.tile([128, JC, RCH], fp16)
            ps_u = [psum.tile([128, RCH], fp32, tag="ps_q", bufs=4,
                              name=f"ps_u{jc}")
                    for jc in range(JC)]
            for tch in range(JC):         # tch-major: both jc advance per W
                for jc in range(JC):
                    nc.tensor.matmul(
                        ps_u[jc],
                        lhsT=cmt[:, tch, jc * 128:(jc + 1) * 128],
                        rhs=w_sb[:, tch, :],
                        start=(tch == 0),
                        stop=(tch == JC - 1),
                    )
                dummies(2)
            v_k(3, False, True)
            for jc in range(JC):
                nc.vector.tensor_mul(
                    a_sb[:, jc, :], ps_u[jc], et_sb[:, jc, :]
                )
            v_sb = work.tile([128, JC, C], fp16)
            for jc in range(JC):
                nc.vector.tensor_copy(
                    out=v_sb[:, jc, 0:256], in_=ps_v[jc][:, 0:256])
                nc.scalar.copy(
                    out=v_sb[:, jc, 256:512], in_=ps_v[jc][:, 256:512])

            # ---- out[r 128, c 512] = A^T V (bv adds on the host), fp16,
            # in 2 column halves so the first DMA overlaps the second.
            o_sb = work.tile([128, C], fp16)
            for ch in range(2):
                cs = ch * (C // 2)
                ps_o = psum.tile([128, C // 2], fp32, tag="ps_o", bufs=2,
                                 name=f"ps_o{ch}")
                for jc in range(JC):
                    nc.tensor.matmul(
                        ps_o,
                        lhsT=a_sb[:, jc, :],
                        rhs=v_sb[:, jc, cs:cs + C // 2],
                        start=(jc == 0),
                        stop=(jc == JC - 1),
                    )
                if ch == 0:
                    nc.vector.tensor_copy(
                        out=o_sb[:, cs:cs + C // 2], in_=ps_o)
                    nc.sync.dma_start(
                        out=out_d[:, cs:cs + C // 2],
                        in_=o_sb[:, cs:cs + C // 2])
                else:
                    nc.scalar.copy(out=o_sb[:, cs:cs + C // 2], in_=ps_o)
                    nc.scalar.dma_start(
                        out=out_d[:, cs:cs + C // 2],
                        in_=o_sb[:, cs:cs + C // 2])
